# revision 14
# baseline (speedup 1.0000x reference)
# Fused dynamic-conv (CondInst-style) + dice loss kernel for 8x TRN2 NeuronCores.
#
# Reference computation (per batch image b, object o):
#   weight[b,o,:] = conv_weight[b, :, ind[b,o]]           (gather, 593 params)
#   feat = concat(seg_feat[b], x_rel(o), y_rel(o))        ([18, 128*128])
#   h1 = relu(w1 @ feat + b1); h2 = relu(w2 @ h1 + b2)    (16-ch dynamic 1x1 convs)
#   out = sigmoid(w3 . h2 + b3)                           ([128*128])
#   dice over masked objects -> scalar loss
#
# Strategy (v4): half-wave (2-tile / 1024 px) software pipeline with
# dedicated PSUM banks per stage:
#   ps1 (banks 0-3): h1 psum [128,1024] double-buffered
#   ps2 (banks 4-5): h2 psum [128,1024]
#   psp (banks 6-7): pred [128,512] double-buffered per dice batch
# Iteration t issues gemm1(t), evac1(t) [DVE], gemm2(t-1), evac2(t-1) [ACT],
# gemm3(t-2) [deferred so the PE FIFO never stalls on evac2], sigmoid at
# batch close, dice (DVE stt + ACT square) deferred one more iteration.
# All inputs ride FOUR consolidated wide DMAs (DIRECT2D trigger cost is per
# DMA, not per byte), and a PE warm-up burst during the fill flips HAM to
# full clock before the real matmuls arrive.
import math
import numpy as np
from contextlib import ExitStack

import concourse.bass as bass
import concourse.tile as tile
from concourse import mybir, bacc
from concourse.bass_utils import run_bass_kernel_spmd

C = 16
WT = 593
B, O, H, W = 4, 32, 128, 128
HW = H * W
N_CORES = 8
GRP = 8            # objects per group (4 pairs)
PXT = 512          # pixels per tile
TPW = 4            # tiles per core per group
WPG = HW // (PXT * TPW)  # = 8 == N_CORES

F32 = mybir.dt.float32
F16 = mybir.dt.float16
ACTF = mybir.ActivationFunctionType
ALU = mybir.AluOpType

WCOL = 288


def host_pack(seg_feat, conv_weight, mask, ind, target):
    cw = conv_weight.reshape(B, WT, HW)
    weight = np.take_along_axis(cw, ind[:, None, :].astype(np.int64), axis=2)
    weight = np.ascontiguousarray(weight.transpose(0, 2, 1))  # [B, O, WT]
    s0 = (C + 2) * C
    w1 = weight[..., :s0].reshape(B, O, C, C + 2)
    b1 = weight[..., s0:s0 + C]
    w2 = weight[..., s0 + C:s0 + C + C * C].reshape(B, O, C, C)
    b2 = weight[..., s0 + C + C * C:s0 + 2 * C + C * C]
    w3 = weight[..., s0 + 2 * C + C * C:s0 + 3 * C + C * C]
    b3 = weight[..., -1]
    xo = (ind % W).astype(np.float32)
    yo = (ind // W).astype(np.float32)

    groups = []  # (b, [obj ids padded with -1])
    for b in range(B):
        objs = [o for o in range(O) if mask[b, o] == 1]
        for g0 in range(0, len(objs), GRP):
            grp = objs[g0:g0 + GRP]
            groups.append((b, grp + [-1] * (GRP - len(grp))))
    if not groups:
        groups.append((0, [-1] * GRP))
    G = len(groups)
    NB = (G + 3) // 4
    KLAST = G - 4 * (NB - 1)

    px = np.arange(HW, dtype=np.float32)
    xg = (px % W) / 128.0
    yg = np.floor(px / W) / 128.0
    tgt_flat = target.reshape(B, O, HW)

    wall = np.zeros((G, 128, WCOL), np.float16)
    ball = np.zeros((G, 128, 2), np.float32)
    for g, (b, grp) in enumerate(groups):
        for oo, o in enumerate(grp):
            if o < 0:
                continue
            pr, sub = oo // 2, oo % 2
            blk = w1[b, o].T.astype(np.float16)  # [18, 16]
            for r in range(4):
                wall[g, 32 * r:32 * r + 18, 32 * pr + 16 * sub:
                     32 * pr + 16 * sub + 16] = blk
            wall[g, 32 * pr + 16 * sub:32 * pr + 16 * sub + 16,
                 128 + 16 * sub:128 + 16 * sub + 16] = \
                w2[b, o].T.astype(np.float16)
            b1e = (b1[b, o] - w1[b, o, :, 16] * (xo[b, o] / 128.0)
                   - w1[b, o, :, 17] * (yo[b, o] / 128.0))
            ball[g, 16 * oo:16 * oo + 16, 0] = b1e
            ball[g, 16 * oo:16 * oo + 16, 1] = b2[b, o]
    for g, (b, grp) in enumerate(groups):
        for oo, o in enumerate(grp):
            if o < 0:
                continue
            for r in range(4):
                wall[g, 16 * oo:16 * oo + 16,
                     160 + 32 * r + 8 * r + oo] = w3[b, o].astype(np.float16)

    wall_t = np.ascontiguousarray(wall.transpose(1, 0, 2)).reshape(128, G * WCOL)
    ball_t = np.ascontiguousarray(ball.transpose(1, 0, 2)).reshape(128, G * 2)

    in_maps = []
    meta = {"groups": groups, "G": G, "NB": NB, "KLAST": KLAST}
    for ci in range(N_CORES):
        feat_pack = np.zeros((G, 128, PXT), np.float16)
        tgt_pack = np.zeros((NB, 128, PXT), np.float32)
        b3_pack = np.full((128, NB), -50.0, np.float32)
        for g, (b, grp) in enumerate(groups):
            for r in range(TPW):
                t = TPW * ci + r
                sl = slice(t * PXT, (t + 1) * PXT)
                feat_pack[g, 32 * r:32 * r + 16] = seg_feat[b].reshape(C, HW)[:, sl]
                feat_pack[g, 32 * r + 16] = xg[sl]
                feat_pack[g, 32 * r + 17] = yg[sl]
            nb, q = g // 4, g % 4
            for r in range(TPW):
                t = TPW * ci + r
                for oo, o in enumerate(grp):
                    if o < 0:
                        continue
                    row = 32 * q + 8 * r + oo
                    tgt_pack[nb, row] = tgt_flat[b, o, t * PXT:(t + 1) * PXT
                                                 ].astype(np.float32)
                    b3_pack[row, nb] = b3[b, o]
        feat2 = np.ascontiguousarray(feat_pack.transpose(1, 0, 2)
                                     ).reshape(128, G * PXT)
        tgt2 = np.ascontiguousarray(tgt_pack.transpose(1, 0, 2)
                                    ).reshape(128, NB * PXT)
        bb = np.ascontiguousarray(np.concatenate([ball_t, b3_pack], axis=1))
        in_maps.append({"feat": feat2, "wall": wall_t, "bb": bb, "tgt": tgt2})
    return in_maps, meta


_PROGRAM_CACHE = {}


def build_program(G, NB, KLAST):
    key = (G, NB, KLAST)
    if key in _PROGRAM_CACHE:
        return _PROGRAM_CACHE[key]
    nc = bacc.Bacc("TRN2", target_bir_lowering=False, debug=False,
                   enable_asserts=False, num_devices=N_CORES)
    feat_t = nc.dram_tensor("feat", (128, G * PXT), F16, kind="ExternalInput")
    wall_t = nc.dram_tensor("wall", (128, G * WCOL), F16, kind="ExternalInput")
    bb_t = nc.dram_tensor("bb", (128, 2 * G + NB), F32, kind="ExternalInput")
    tgt_t = nc.dram_tensor("tgt", (128, NB * PXT), F32, kind="ExternalInput")
    acc_t = nc.dram_tensor("acc", (128, 2 * NB), F32, kind="ExternalOutput")

    NW = 2 * G  # half-waves

    with tile.TileContext(nc) as tc, ExitStack() as ctx:
        wpool = ctx.enter_context(tc.tile_pool(name="wpool", bufs=1))
        h1pool = ctx.enter_context(tc.tile_pool(name="h1pool", bufs=3))
        h2pool = ctx.enter_context(tc.tile_pool(name="h2pool", bufs=10))
        spool = ctx.enter_context(tc.tile_pool(name="spool", bufs=2))
        apool = ctx.enter_context(tc.tile_pool(name="apool", bufs=1))
        ps1 = ctx.enter_context(tc.tile_pool(name="ps1", bufs=2, space="PSUM"))
        ps2 = ctx.enter_context(tc.tile_pool(name="ps2", bufs=2, space="PSUM"))

        acc_sb = apool.tile([128, 2 * NB], F32)
        inter_acc = acc_sb[:, 0:NB]
        psq_acc = acc_sb[:, NB:2 * NB]

        # ACT table warm first on the scalar queue (so its auto table load
        # runs during the fill), then input DMAs.  Separate tiles per DMA
        # chunk so consumers gate only on the chunk they need.
        scr = apool.tile([128, 256], F16)
        nc.vector.memset(scr[0:8, :], 0.125)
        scr1 = apool.tile([128, 1], F32)
        nc.scalar.activation(scr1[0:1, :], scr[0:1, 0:1], ACTF.Sigmoid,
                             bias=0.0, scale=1.0)

        w0sb = wpool.tile([128, WCOL], F16)
        wRsb = wpool.tile([128, max(G - 1, 1) * WCOL], F16)
        f_tiles = {}
        f0sb = wpool.tile([128, PXT], F16)
        bbsb = wpool.tile([128, 2 * G + NB], F32)
        tgtsb = wpool.tile([128, NB * PXT], F32)
        nc.sync.dma_start(out=w0sb, in_=wall_t.ap()[:, 0:WCOL])
        nc.sync.dma_start(out=f0sb, in_=feat_t.ap()[:, 0:PXT])
        nc.scalar.dma_start(out=bbsb, in_=bb_t.ap())
        if G > 1:
            f1sb = wpool.tile([128, PXT], F16)
            nc.scalar.dma_start(out=f1sb, in_=feat_t.ap()[:, PXT:2 * PXT])
            f_tiles[1] = (f1sb, 0)
            nc.sync.dma_start(out=wRsb,
                              in_=wall_t.ap()[:, WCOL:G * WCOL])
        if G > 2:
            n23 = min(G, 4) - 2
            f23sb = wpool.tile([128, n23 * PXT], F16)
            nc.sync.dma_start(out=f23sb,
                              in_=feat_t.ap()[:, 2 * PXT:(2 + n23) * PXT])
            for g in range(2, 2 + n23):
                f_tiles[g] = (f23sb, (g - 2) * PXT)
        if G > 4:
            fRsb = wpool.tile([128, (G - 4) * PXT], F16)
            nc.sync.dma_start(out=fRsb,
                              in_=feat_t.ap()[:, 4 * PXT:G * PXT])
            for g in range(4, G):
                f_tiles[g] = (fRsb, (g - 4) * PXT)
        nc.scalar.dma_start(out=tgtsb, in_=tgt_t.ap())
        f_tiles[0] = (f0sb, 0)

        def wslice(g, lo, hi):
            if g == 0:
                return w0sb[:, lo:hi]
            return wRsb[:, WCOL * (g - 1) + lo:WCOL * (g - 1) + hi]

        def fslice(g):
            tile_, off = f_tiles[g]
            return tile_[:, off:off + PXT]

        # PE warm-up burst so HAM is at full clock when real matmuls arrive
        wu = ps2.tile([128, 1024], F32, tag="p2", name="warm")
        for i in range(6):
            nc.tensor.matmul(wu[:, 0:256], scr[0:8, 0:128], scr[0:8, :],
                             start=True, stop=True)

        def gemm1(t):
            g = t // 2
            wv = t % 2
            p1 = ps1.tile([128, 1024], F32, tag="p1", name=f"p1_{t}")
            for rr in range(2):
                r = 2 * wv + rr
                for c2 in range(4):
                    nc.tensor.matmul(
                        p1[32 * c2:32 * c2 + 32, 512 * rr:512 * rr + 512],
                        wslice(g, 0, 128)[32 * r:32 * r + 18,
                                          32 * c2:32 * c2 + 32],
                        fslice(g)[32 * r:32 * r + 18, :],
                        start=True, stop=True,
                        tile_position=(32 * r, 32 * c2))
            return p1

        h1_of = {}
        h2_of = {}
        pred_of = {}
        p1_of = {0: gemm1(0)}
        pending_dice = []

        for t in range(NW + 2):
            # evac1(t) on DVE, then issue gemm1(t+1) so the PE FIFO's
            # p1-ring wait IS the pipeline wait (PE runs one wave ahead)
            if t < NW:
                g = t // 2
                p1 = p1_of.pop(t)
                h1sb_t = h1pool.tile([128, 1024], F16, tag="h1", name=f"h1_{t}")
                nc.vector.tensor_scalar(
                    out=h1sb_t, in0=p1, scalar1=bbsb[:, 2 * g:2 * g + 1],
                    scalar2=0.0, op0=ALU.add, op1=ALU.max)
                h1_of[t] = h1sb_t
                if t + 1 < NW:
                    p1_of[t + 1] = gemm1(t + 1)

            # gemm2(t-1) + evac2(t-1) on ACT
            if 1 <= t <= NW:
                v = t - 1
                vg = v // 2
                h1sb_v = h1_of.pop(v)
                p2 = ps2.tile([128, 1024], F32, tag="p2", name=f"p2_{v}")
                for h in range(2):
                    for x in range(4):
                        nc.tensor.matmul(
                            p2[32 * x:32 * x + 32, 512 * h:512 * h + 512],
                            wslice(vg, 128, 160)[32 * x:32 * x + 32, :],
                            h1sb_v[32 * x:32 * x + 32, 512 * h:512 * h + 512],
                            start=True, stop=True,
                            tile_position=(32 * x, 32 * x))
                h2sb_v = h2pool.tile([128, 1024], F16, tag="h2", name=f"h2_{v}")
                nc.scalar.activation(h2sb_v, p2, ACTF.Relu,
                                     bias=bbsb[:, 2 * vg + 1:2 * vg + 2],
                                     scale=1.0)
                h2_of[v] = h2sb_v

                # batch close: batched gemm3 bursts (pred borrows a ps1 ring
                # slot; the one-wave PE lookahead absorbs the borrow).  All
                # but the last quadrant burst one iteration early so the
                # sigmoid trails only 4 MMs.
                ug = vg
                nb = ug // 4
                q = ug % 4
                klast_q = KLAST if nb == NB - 1 else 4

                def g3_quad(pred, nb, qq):
                    gq = 4 * nb + qq
                    for r in range(4):
                        u = 2 * gq + r // 2
                        nc.tensor.matmul(
                            pred[32 * qq:32 * qq + 32, :],
                            wslice(gq, 160 + 32 * r, 192 + 32 * r),
                            h2_of[u][:, 512 * (r % 2):512 * (r % 2) + 512],
                            start=(r == 0), stop=(r == 3),
                            tile_position=(0, 32 * qq))

                # early part only for the LAST batch (no later p1 allocs can
                # slip between the pred borrow and its readers there)
                last_b = nb == NB - 1
                if (last_b and klast_q >= 2 and q == klast_q - 2
                        and v % 2 == 1):
                    pred = ps1.tile([128, PXT], F32, tag="p1",
                                    name=f"pred{nb}")
                    pred_of[nb] = pred
                    for qq in range(klast_q - 1):
                        g3_quad(pred, nb, qq)
                    for gq in range(4 * nb, 4 * nb + klast_q - 1):
                        h2_of.pop(2 * gq)
                        h2_of.pop(2 * gq + 1)
                if q == klast_q - 1 and v % 2 == 1:
                    if last_b and klast_q >= 2:
                        pred = pred_of[nb]
                        first_q = klast_q - 1
                    else:
                        pred = ps1.tile([128, PXT], F32, tag="p1",
                                        name=f"pred{nb}")
                        first_q = 0
                    for qq in range(first_q, klast_q):
                        g3_quad(pred, nb, qq)
                    for gq in range(4 * nb + first_q, 4 * nb + klast_q):
                        h2_of.pop(2 * gq)
                        h2_of.pop(2 * gq + 1)
                    pp = 32 * klast_q
                    predsb = spool.tile([128, PXT], F32, tag="psb",
                                        name=f"psb{nb}")
                    nc.scalar.activation(predsb[0:pp, :], pred[0:pp, :],
                                         ACTF.Sigmoid,
                                         bias=bbsb[0:pp, 2 * G + nb:
                                                   2 * G + nb + 1],
                                         scale=1.0)
                    pending_dice.append((t, nb, predsb, pp))

            # deferred dice: two iterations after the sigmoid issue so the
            # evac stream never queue-stalls behind the sigmoid chain
            while pending_dice and (t == NW + 1 or pending_dice[0][0] < t - 1):
                _, nb_d, predsb, pp = pending_dice.pop(0)
                tgv = tgtsb[:, PXT * nb_d:PXT * (nb_d + 1)]
                sc1 = spool.tile([128, PXT], F32, tag="s1", name=f"sc1{nb_d}")
                sc2 = spool.tile([128, PXT], F32, tag="s2", name=f"sc2{nb_d}")
                nc.vector.scalar_tensor_tensor(
                    out=sc1[0:pp, :], in0=predsb[0:pp, :], scalar=0.0,
                    in1=tgv[0:pp, :], op0=ALU.add, op1=ALU.mult,
                    accum_out=inter_acc[0:pp, nb_d:nb_d + 1])
                nc.scalar.activation(
                    sc2[0:pp, :], predsb[0:pp, :], ACTF.Square,
                    accum_out=psq_acc[0:pp, nb_d:nb_d + 1])

        nc.sync.dma_start(out=acc_t.ap(), in_=acc_sb)

    nc.compile()
    _PROGRAM_CACHE[key] = nc
    return nc


def _run(inputs, trace=False):
    seg_feat = np.asarray(inputs["seg_feat"], np.float32)
    conv_weight = np.asarray(inputs["conv_weight"], np.float32)
    mask = np.asarray(inputs["mask"])
    ind = np.asarray(inputs["ind"])
    target = np.asarray(inputs["target"], np.float32)

    in_maps, meta = host_pack(seg_feat, conv_weight, mask, ind, target)
    G, NB, KLAST = meta["G"], meta["NB"], meta["KLAST"]
    groups = meta["groups"]
    nc = build_program(G, NB, KLAST)
    res = run_bass_kernel_spmd(nc, in_maps, core_ids=list(range(N_CORES)),
                               trace=trace)

    inter = np.zeros(B, np.float64)
    predsq = np.zeros(B, np.float64)
    for ci in range(N_CORES):
        acc = res.results[ci]["acc"]
        for g, (b, grp) in enumerate(groups):
            if all(o < 0 for o in grp):
                continue
            nb, q = g // 4, g % 4
            inter[b] += acc[32 * q:32 * q + 32, nb].sum(dtype=np.float64)
            predsq[b] += acc[32 * q:32 * q + 32, NB + nb].sum(dtype=np.float64)
    tgtsq = ((target.reshape(B, O, HW).astype(np.float64) ** 2)
             * mask[:, :, None]).sum(axis=(1, 2))
    loss = 1.0 - (2.0 * inter + 1.0) / (predsq + tgtsq + 1.0)
    return np.float32(loss.mean()), res


def kernel(**inputs):
    loss, _ = _run(inputs, trace=False)
    return np.array(loss, dtype=np.float32)


# revision 16
# speedup vs baseline: 1.0190x; 1.0190x over previous
# Fused dynamic-conv (CondInst-style) + dice loss kernel for 8x TRN2 NeuronCores.
#
# Reference computation (per batch image b, object o):
#   weight[b,o,:] = conv_weight[b, :, ind[b,o]]           (gather, 593 params)
#   feat = concat(seg_feat[b], x_rel(o), y_rel(o))        ([18, 128*128])
#   h1 = relu(w1 @ feat + b1); h2 = relu(w2 @ h1 + b2)    (16-ch dynamic 1x1 convs)
#   out = sigmoid(w3 . h2 + b3)                           ([128*128])
#   dice over masked objects -> scalar loss
#
# Strategy (v4): half-wave (2-tile / 1024 px) software pipeline with
# dedicated PSUM banks per stage:
#   ps1 (banks 0-3): h1 psum [128,1024] double-buffered
#   ps2 (banks 4-5): h2 psum [128,1024]
#   psp (banks 6-7): pred [128,512] double-buffered per dice batch
# Iteration t issues gemm1(t), evac1(t) [DVE], gemm2(t-1), evac2(t-1) [ACT],
# gemm3(t-2) [deferred so the PE FIFO never stalls on evac2], sigmoid at
# batch close, dice (DVE stt + ACT square) deferred one more iteration.
# All inputs ride FOUR consolidated wide DMAs (DIRECT2D trigger cost is per
# DMA, not per byte), and a PE warm-up burst during the fill flips HAM to
# full clock before the real matmuls arrive.
import math
import numpy as np
from contextlib import ExitStack

import concourse.bass as bass
import concourse.tile as tile
from concourse import mybir, bacc
from concourse.bass_utils import run_bass_kernel_spmd

C = 16
WT = 593
B, O, H, W = 4, 32, 128, 128
HW = H * W
N_CORES = 8
GRP = 8            # objects per group (4 pairs)
PXT = 512          # pixels per tile
TPW = 4            # tiles per core per group
WPG = HW // (PXT * TPW)  # = 8 == N_CORES

F32 = mybir.dt.float32
F16 = mybir.dt.float16
ACTF = mybir.ActivationFunctionType
ALU = mybir.AluOpType

WCOL = 288


def host_pack(seg_feat, conv_weight, mask, ind, target):
    cw = conv_weight.reshape(B, WT, HW)
    weight = np.take_along_axis(cw, ind[:, None, :].astype(np.int64), axis=2)
    weight = np.ascontiguousarray(weight.transpose(0, 2, 1))  # [B, O, WT]
    s0 = (C + 2) * C
    w1 = weight[..., :s0].reshape(B, O, C, C + 2)
    b1 = weight[..., s0:s0 + C]
    w2 = weight[..., s0 + C:s0 + C + C * C].reshape(B, O, C, C)
    b2 = weight[..., s0 + C + C * C:s0 + 2 * C + C * C]
    w3 = weight[..., s0 + 2 * C + C * C:s0 + 3 * C + C * C]
    b3 = weight[..., -1]
    xo = (ind % W).astype(np.float32)
    yo = (ind // W).astype(np.float32)

    groups = []  # (b, [obj ids padded with -1])
    for b in range(B):
        objs = [o for o in range(O) if mask[b, o] == 1]
        for g0 in range(0, len(objs), GRP):
            grp = objs[g0:g0 + GRP]
            groups.append((b, grp + [-1] * (GRP - len(grp))))
    if not groups:
        groups.append((0, [-1] * GRP))
    G = len(groups)
    NB = (G + 3) // 4
    KLAST = G - 4 * (NB - 1)

    px = np.arange(HW, dtype=np.float32)
    xg = (px % W) / 128.0
    yg = np.floor(px / W) / 128.0
    tgt_flat = target.reshape(B, O, HW)

    wall = np.zeros((G, 128, WCOL), np.float16)
    ball = np.zeros((G, 128, 2), np.float32)
    for g, (b, grp) in enumerate(groups):
        for oo, o in enumerate(grp):
            if o < 0:
                continue
            pr, sub = oo // 2, oo % 2
            blk = w1[b, o].T.astype(np.float16)  # [18, 16]
            for r in range(4):
                wall[g, 32 * r:32 * r + 18, 32 * pr + 16 * sub:
                     32 * pr + 16 * sub + 16] = blk
            wall[g, 32 * pr + 16 * sub:32 * pr + 16 * sub + 16,
                 128 + 16 * sub:128 + 16 * sub + 16] = \
                w2[b, o].T.astype(np.float16)
            b1e = (b1[b, o] - w1[b, o, :, 16] * (xo[b, o] / 128.0)
                   - w1[b, o, :, 17] * (yo[b, o] / 128.0))
            ball[g, 16 * oo:16 * oo + 16, 0] = b1e
            ball[g, 16 * oo:16 * oo + 16, 1] = b2[b, o]
    for g, (b, grp) in enumerate(groups):
        for oo, o in enumerate(grp):
            if o < 0:
                continue
            for r in range(4):
                wall[g, 16 * oo:16 * oo + 16,
                     160 + 32 * r + 8 * r + oo] = w3[b, o].astype(np.float16)

    wall_t = np.ascontiguousarray(wall.transpose(1, 0, 2)).reshape(128, G * WCOL)
    ball_t = np.ascontiguousarray(ball.transpose(1, 0, 2)).reshape(128, G * 2)

    in_maps = []
    meta = {"groups": groups, "G": G, "NB": NB, "KLAST": KLAST}
    for ci in range(N_CORES):
        feat_pack = np.zeros((G, 128, PXT), np.float16)
        tgt_pack = np.zeros((NB, 128, PXT), np.float32)
        b3_pack = np.full((128, NB), -50.0, np.float32)
        for g, (b, grp) in enumerate(groups):
            for r in range(TPW):
                t = TPW * ci + r
                sl = slice(t * PXT, (t + 1) * PXT)
                feat_pack[g, 32 * r:32 * r + 16] = seg_feat[b].reshape(C, HW)[:, sl]
                feat_pack[g, 32 * r + 16] = xg[sl]
                feat_pack[g, 32 * r + 17] = yg[sl]
            nb, q = g // 4, g % 4
            for r in range(TPW):
                t = TPW * ci + r
                for oo, o in enumerate(grp):
                    if o < 0:
                        continue
                    row = 32 * q + 8 * r + oo
                    tgt_pack[nb, row] = tgt_flat[b, o, t * PXT:(t + 1) * PXT
                                                 ].astype(np.float32)
                    b3_pack[row, nb] = b3[b, o]
        feat2 = np.ascontiguousarray(feat_pack.transpose(1, 0, 2)
                                     ).reshape(128, G * PXT)
        tgt2 = np.ascontiguousarray(tgt_pack.transpose(1, 0, 2)
                                    ).reshape(128, NB * PXT)
        bb = np.ascontiguousarray(np.concatenate([ball_t, b3_pack], axis=1))
        in_maps.append({"feat": feat2, "wall": wall_t, "bb": bb, "tgt": tgt2})
    return in_maps, meta


_PROGRAM_CACHE = {}


def build_program(G, NB, KLAST):
    key = (G, NB, KLAST)
    if key in _PROGRAM_CACHE:
        return _PROGRAM_CACHE[key]
    nc = bacc.Bacc("TRN2", target_bir_lowering=False, debug=False,
                   enable_asserts=False, num_devices=N_CORES)
    feat_t = nc.dram_tensor("feat", (128, G * PXT), F16, kind="ExternalInput")
    wall_t = nc.dram_tensor("wall", (128, G * WCOL), F16, kind="ExternalInput")
    bb_t = nc.dram_tensor("bb", (128, 2 * G + NB), F32, kind="ExternalInput")
    tgt_t = nc.dram_tensor("tgt", (128, NB * PXT), F32, kind="ExternalInput")
    acc_t = nc.dram_tensor("acc", (128, 2 * NB), F32, kind="ExternalOutput")

    NW = 2 * G  # half-waves

    with tile.TileContext(nc) as tc, ExitStack() as ctx:
        wpool = ctx.enter_context(tc.tile_pool(name="wpool", bufs=1))
        h1pool = ctx.enter_context(tc.tile_pool(name="h1pool", bufs=3))
        h2pool = ctx.enter_context(tc.tile_pool(name="h2pool", bufs=10))
        spool = ctx.enter_context(tc.tile_pool(name="spool", bufs=2))
        apool = ctx.enter_context(tc.tile_pool(name="apool", bufs=1))
        ps1 = ctx.enter_context(tc.tile_pool(name="ps1", bufs=2, space="PSUM"))
        ps2 = ctx.enter_context(tc.tile_pool(name="ps2", bufs=2, space="PSUM"))

        acc_sb = apool.tile([128, 2 * NB], F32)
        inter_acc = acc_sb[:, 0:NB]
        psq_acc = acc_sb[:, NB:2 * NB]

        # Input DMAs: separate tiles per chunk so consumers gate only on the
        # chunk they need.  sync queue: weights + most feat; scalar queue:
        # bias + one feat chunk + targets (triggers first, before the auto
        # ACT table loads, so data lands during the fill).
        w0sb = wpool.tile([128, WCOL], F16)
        wRsb = wpool.tile([128, max(G - 1, 1) * WCOL], F16)
        f_tiles = {}
        f0sb = wpool.tile([128, PXT], F16)
        bbsb = wpool.tile([128, 2 * G + NB], F32)
        tgtsb = wpool.tile([128, NB * PXT], F32)
        nc.sync.dma_start(out=w0sb, in_=wall_t.ap()[:, 0:WCOL])
        nc.sync.dma_start(out=f0sb, in_=feat_t.ap()[:, 0:PXT])
        nc.scalar.dma_start(out=bbsb, in_=bb_t.ap())
        if G > 1:
            f1sb = wpool.tile([128, PXT], F16)
            nc.scalar.dma_start(out=f1sb, in_=feat_t.ap()[:, PXT:2 * PXT])
            f_tiles[1] = (f1sb, 0)
            nc.sync.dma_start(out=wRsb,
                              in_=wall_t.ap()[:, WCOL:G * WCOL])
        if G > 2:
            n23 = min(G, 4) - 2
            f23sb = wpool.tile([128, n23 * PXT], F16)
            nc.sync.dma_start(out=f23sb,
                              in_=feat_t.ap()[:, 2 * PXT:(2 + n23) * PXT])
            for g in range(2, 2 + n23):
                f_tiles[g] = (f23sb, (g - 2) * PXT)
        if G > 4:
            fRsb = wpool.tile([128, (G - 4) * PXT], F16)
            nc.sync.dma_start(out=fRsb,
                              in_=feat_t.ap()[:, 4 * PXT:G * PXT])
            for g in range(4, G):
                f_tiles[g] = (fRsb, (g - 4) * PXT)
        f_tiles[0] = (f0sb, 0)

        # ACT table warm (auto table loads precede this dummy activation)
        scr = apool.tile([128, 256], F16)
        nc.vector.memset(scr[0:8, :], 0.125)
        scr1 = apool.tile([128, 1], F32)
        nc.scalar.activation(scr1[0:1, :], scr[0:1, 0:1], ACTF.Sigmoid,
                             bias=0.0, scale=1.0)
        nc.scalar.dma_start(out=tgtsb, in_=tgt_t.ap())

        def wslice(g, lo, hi):
            if g == 0:
                return w0sb[:, lo:hi]
            return wRsb[:, WCOL * (g - 1) + lo:WCOL * (g - 1) + hi]

        def fslice(g):
            tile_, off = f_tiles[g]
            return tile_[:, off:off + PXT]

        # PE warm-up burst so HAM is at full clock when real matmuls arrive
        wu = ps2.tile([128, 1024], F32, tag="p2", name="warm")
        for i in range(6):
            nc.tensor.matmul(wu[:, 0:256], scr[0:8, 0:128], scr[0:8, :],
                             start=True, stop=True)

        def gemm1(t):
            g = t // 2
            wv = t % 2
            p1 = ps1.tile([128, 1024], F32, tag="p1", name=f"p1_{t}")
            for rr in range(2):
                r = 2 * wv + rr
                for c2 in range(4):
                    nc.tensor.matmul(
                        p1[32 * c2:32 * c2 + 32, 512 * rr:512 * rr + 512],
                        wslice(g, 0, 128)[32 * r:32 * r + 18,
                                          32 * c2:32 * c2 + 32],
                        fslice(g)[32 * r:32 * r + 18, :],
                        start=True, stop=True,
                        tile_position=(32 * r, 32 * c2))
            return p1

        h1_of = {}
        h2_of = {}
        pred_of = {}
        p1_of = {0: gemm1(0)}
        pending_dice = []

        for t in range(NW + 2):
            # evac1(t) on DVE, then issue gemm1(t+1) so the PE FIFO's
            # p1-ring wait IS the pipeline wait (PE runs one wave ahead)
            if t < NW:
                g = t // 2
                p1 = p1_of.pop(t)
                h1sb_t = h1pool.tile([128, 1024], F16, tag="h1", name=f"h1_{t}")
                nc.vector.tensor_scalar(
                    out=h1sb_t, in0=p1, scalar1=bbsb[:, 2 * g:2 * g + 1],
                    scalar2=0.0, op0=ALU.add, op1=ALU.max)
                h1_of[t] = h1sb_t
                if t + 1 < NW:
                    p1_of[t + 1] = gemm1(t + 1)

            # gemm2(t-1) + evac2(t-1) on ACT
            if 1 <= t <= NW:
                v = t - 1
                vg = v // 2
                h1sb_v = h1_of.pop(v)
                p2 = ps2.tile([128, 1024], F32, tag="p2", name=f"p2_{v}")
                for h in range(2):
                    for x in range(4):
                        nc.tensor.matmul(
                            p2[32 * x:32 * x + 32, 512 * h:512 * h + 512],
                            wslice(vg, 128, 160)[32 * x:32 * x + 32, :],
                            h1sb_v[32 * x:32 * x + 32, 512 * h:512 * h + 512],
                            start=True, stop=True,
                            tile_position=(32 * x, 32 * x))
                h2sb_v = h2pool.tile([128, 1024], F16, tag="h2", name=f"h2_{v}")
                nc.scalar.activation(h2sb_v, p2, ACTF.Relu,
                                     bias=bbsb[:, 2 * vg + 1:2 * vg + 2],
                                     scale=1.0)
                h2_of[v] = h2sb_v

                # batch close: batched gemm3 bursts (pred borrows a ps1 ring
                # slot; the one-wave PE lookahead absorbs the borrow).  All
                # but the last quadrant burst one iteration early so the
                # sigmoid trails only 4 MMs.
                ug = vg
                nb = ug // 4
                q = ug % 4
                klast_q = KLAST if nb == NB - 1 else 4

                def g3_quad(pred, nb, qq):
                    gq = 4 * nb + qq
                    for r in range(4):
                        u = 2 * gq + r // 2
                        nc.tensor.matmul(
                            pred[32 * qq:32 * qq + 32, :],
                            wslice(gq, 160 + 32 * r, 192 + 32 * r),
                            h2_of[u][:, 512 * (r % 2):512 * (r % 2) + 512],
                            start=(r == 0), stop=(r == 3),
                            tile_position=(0, 32 * qq))

                # quadrants 0..k-2 burst one iteration before the close (the
                # pred borrow slots into the ps1 ring so its readers all
                # precede the ring slot's next reallocation)
                if klast_q >= 2 and q == klast_q - 1 and v % 2 == 0:
                    pred = ps1.tile([128, PXT], F32, tag="p1",
                                    name=f"pred{nb}")
                    pred_of[nb] = pred
                    for qq in range(klast_q - 1):
                        g3_quad(pred, nb, qq)
                    for gq in range(4 * nb, 4 * nb + klast_q - 1):
                        h2_of.pop(2 * gq)
                        h2_of.pop(2 * gq + 1)
                if q == klast_q - 1 and v % 2 == 1:
                    if klast_q >= 2:
                        pred = pred_of[nb]
                        first_q = klast_q - 1
                    else:
                        pred = ps1.tile([128, PXT], F32, tag="p1",
                                        name=f"pred{nb}")
                        first_q = 0
                    for qq in range(first_q, klast_q):
                        g3_quad(pred, nb, qq)
                    for gq in range(4 * nb + first_q, 4 * nb + klast_q):
                        h2_of.pop(2 * gq)
                        h2_of.pop(2 * gq + 1)
                    pp = 32 * klast_q
                    predsb = spool.tile([128, PXT], F32, tag="psb",
                                        name=f"psb{nb}")
                    nc.scalar.activation(predsb[0:pp, :], pred[0:pp, :],
                                         ACTF.Sigmoid,
                                         bias=bbsb[0:pp, 2 * G + nb:
                                                   2 * G + nb + 1],
                                         scale=1.0)
                    pending_dice.append((t, nb, predsb, pp))

            # deferred dice: two iterations after the sigmoid issue so the
            # evac stream never queue-stalls behind the sigmoid chain
            while pending_dice and (t == NW + 1 or pending_dice[0][0] < t - 1):
                _, nb_d, predsb, pp = pending_dice.pop(0)
                tgv = tgtsb[:, PXT * nb_d:PXT * (nb_d + 1)]
                sc1 = spool.tile([128, PXT], F32, tag="s1", name=f"sc1{nb_d}")
                sc2 = spool.tile([128, PXT], F32, tag="s2", name=f"sc2{nb_d}")
                nc.vector.scalar_tensor_tensor(
                    out=sc1[0:pp, :], in0=predsb[0:pp, :], scalar=0.0,
                    in1=tgv[0:pp, :], op0=ALU.add, op1=ALU.mult,
                    accum_out=inter_acc[0:pp, nb_d:nb_d + 1])
                nc.scalar.activation(
                    sc2[0:pp, :], predsb[0:pp, :], ACTF.Square,
                    accum_out=psq_acc[0:pp, nb_d:nb_d + 1])

        nc.sync.dma_start(out=acc_t.ap(), in_=acc_sb)

    nc.compile()
    _PROGRAM_CACHE[key] = nc
    return nc


def _run(inputs, trace=False):
    seg_feat = np.asarray(inputs["seg_feat"], np.float32)
    conv_weight = np.asarray(inputs["conv_weight"], np.float32)
    mask = np.asarray(inputs["mask"])
    ind = np.asarray(inputs["ind"])
    target = np.asarray(inputs["target"], np.float32)

    in_maps, meta = host_pack(seg_feat, conv_weight, mask, ind, target)
    G, NB, KLAST = meta["G"], meta["NB"], meta["KLAST"]
    groups = meta["groups"]
    nc = build_program(G, NB, KLAST)
    res = run_bass_kernel_spmd(nc, in_maps, core_ids=list(range(N_CORES)),
                               trace=trace)

    inter = np.zeros(B, np.float64)
    predsq = np.zeros(B, np.float64)
    for ci in range(N_CORES):
        acc = res.results[ci]["acc"]
        for g, (b, grp) in enumerate(groups):
            if all(o < 0 for o in grp):
                continue
            nb, q = g // 4, g % 4
            inter[b] += acc[32 * q:32 * q + 32, nb].sum(dtype=np.float64)
            predsq[b] += acc[32 * q:32 * q + 32, NB + nb].sum(dtype=np.float64)
    tgtsq = ((target.reshape(B, O, HW).astype(np.float64) ** 2)
             * mask[:, :, None]).sum(axis=(1, 2))
    loss = 1.0 - (2.0 * inter + 1.0) / (predsq + tgtsq + 1.0)
    return np.float32(loss.mean()), res


def kernel(**inputs):
    loss, _ = _run(inputs, trace=False)
    return np.array(loss, dtype=np.float32)
